# revision 1
# baseline (speedup 1.0000x reference)
"""Trainium2 Bass kernel for nn_CustomModel_74586402063130.

Model: logits = GRU(E[words] + midis@Wm, mask=words!=0) @ Wo
Shapes: B=32, T=256, V=10000, E_DIM=300, M_DIM=128, H=256.

Strategy (8 cores): replicate the embedding+GRU on every core (the
recurrence is latency-bound, not FLOP-bound), shard the output Dense
over the vocab dimension (1250 columns per core). No collectives.

Per-core dataflow (all matmuls bf16, fp32 PSUM, transposed layouts —
feature dims on partitions, tokens on the free axis, token col order
t*32+b):
  xT   [302, 8192]  = gathered E rows (dma_gather transpose) + Wm.T@midisT
                      (+ ones row for folded biases, + (1-mask) row)
  gxT  [768, 8192]  = Wx'.T @ xT   (z-gate columns negated, mask bias and
                      bx/bm folded in via the extra xT rows)
  recurrence over t: PSUM_zr = gx_zr[t] (identity matmul) + U'.T h
                     PSUM_n  = U_n.T h
                     [s|r] = sigmoid(PSUM_zr)        (s = 1-z via negation)
                     n = tanh(r*PSUM_n + gx_n[t])
                     h' = h - s*(h - n)              (pad steps: s ~= 0)
  logits[t-block]   = seq.T-as-weights @ Wo-slice, PSUM->SBUF->DRAM,
                      pipelined into the recurrence every 4 steps.
"""
import numpy as np
import ml_dtypes
from contextlib import ExitStack

B, T, V, E_DIM, M_DIM, H = 32, 256, 10000, 300, 128, 256
NBT = B * T              # 8192 tokens
NCORES = 8
VSH = V // NCORES        # 1250 vocab columns per core
EP = 384                 # padded embedding row (bf16: 768B, %256 for gather)
KX = E_DIM + 2           # 300 features + ones row + mask row
BIG = 30.0               # sigmoid(-30) ~ 1e-13: masked steps keep h exactly
CG = 512                 # token col-group size for the x/gx pipeline
NCG = NBT // CG          # 16 col groups

bf16 = ml_dtypes.bfloat16


def _host_prep(words, midis, E, Wm, bm, Wx, bx, U, bu, Wo, bo):
    """Numpy-side weight/index preparation (O(V*E) transforms only)."""
    f32 = np.float32
    words = np.asarray(words)
    mask = (words != 0)

    # token col order: col = t*32 + b
    iw = np.ascontiguousarray(words.T).reshape(-1)          # [8192] t-major
    # indirect-DMA gather: tile i gathers rows iw[i*128 + p] into partition p
    idx32 = np.ascontiguousarray(
        iw.reshape(NBT // 128, 128).T).astype(np.int32)     # [128, 64]

    e_pad = np.zeros((V, EP), dtype=bf16)
    e_pad[:, :E_DIM] = np.asarray(E, f32).astype(bf16)

    midisT = np.ascontiguousarray(
        np.asarray(midis, f32).transpose(1, 0, 2).reshape(NBT, M_DIM).T
    ).astype(bf16)                                          # [128, 8192]

    onesmask = np.empty((2, NBT), dtype=bf16)
    onesmask[0] = 1.0
    onesmask[1] = (1.0 - np.ascontiguousarray(mask.T).reshape(-1)).astype(bf16)

    wm = np.zeros((M_DIM, 3 * 128), dtype=bf16)             # chunk-major M
    wm[:, :E_DIM] = np.asarray(Wm, f32).astype(bf16)        # cols 300..383: 0

    # Wx' rows: 0..299 = Wx (z cols negated), 300 = bias row, 301 = mask row
    wxp_f = np.zeros((3 * 128, 3 * H), dtype=f32)
    wxp_f[:E_DIM] = np.asarray(Wx, f32)
    wxp_f[E_DIM] = np.asarray(bm, f32) @ np.asarray(Wx, f32) + np.asarray(bx, f32)
    wxp_f[:, :H] *= -1.0                                    # negate z columns
    wxp_f[E_DIM + 1, :H] = -BIG                             # mask row weight
    # also fold bu_z, bu_r into the per-step bias (added every step)
    buf = np.asarray(bu, f32)
    wxp_f[E_DIM, :H] += -buf[:H]
    wxp_f[E_DIM, H:2 * H] += buf[H:2 * H]
    # store chunk-major: [128, 3*768], chunk c = rows c*128..c*128+127
    wxp = np.ascontiguousarray(
        wxp_f.reshape(3, 128, 3 * H).transpose(1, 0, 2).reshape(128, 3 * 3 * H)
    ).astype(bf16)

    up_f = np.asarray(U, f32).copy()
    up_f[:, :H] = -up_f[:, :H]
    up = np.ascontiguousarray(
        up_f.reshape(2, 128, 3 * H).transpose(1, 0, 2).reshape(128, 2 * 3 * H)
    ).astype(bf16)                                          # [128, 2*768]

    bun = np.asarray(bu, f32)[2 * H:]                       # [256] n-gate bias
    bun2 = np.ascontiguousarray(bun.reshape(2, 128).T).astype(f32)  # [128, 2]

    ident = np.eye(128, dtype=bf16)

    wo_f = np.asarray(Wo, f32)                              # [256, V]
    wo_cores = []
    for c in range(NCORES):
        sl = wo_f[:, c * VSH:(c + 1) * VSH]                 # [256, 1250]
        wo_cores.append(np.ascontiguousarray(
            sl.reshape(2, 128, VSH).transpose(1, 0, 2).reshape(128, 2 * VSH)
        ).astype(bf16))

    bo_f = np.asarray(bo, f32)
    return dict(idx32=idx32, e_pad=e_pad, midisT=midisT, onesmask=onesmask,
                wm=wm, wxp=wxp, up=up, bun2=bun2, ident=ident,
                wo_cores=wo_cores, bo_f=bo_f,
                has_bun=bool(np.any(np.asarray(bu, f32)[2 * H:])),
                has_bo=bool(np.any(bo_f)))


def _apply_tile_patch():
    """This container's walrus rejects >1 semaphore wait on a Drain
    instruction; Tile's kernel-tail drain aggregates one wait per active
    sem lane onto a single Drain. Split them across a chain of Drains."""
    import concourse.mybir as mybir
    import concourse.tile as tile
    from concourse.vector_clock import ScopedClock

    if getattr(tile.TileContext, "_drain_split_patched", False):
        return

    def _patched(self, tick_clock, wait_clock):
        drain_inst = self.nc.sync.drain()
        wait_clock.add_sem_waits(
            drain_inst.ins, ScopedClock({None: tick_clock.global_clock})
        )
        inst = drain_inst.ins
        si = inst.sync_info
        if si is not None and len(si.on_wait) > 1:
            waits = list(si.on_wait)
            si.on_wait = waits[:1]
            inst.sync_info = si
            for w in waits[1:]:
                d2 = self.nc.sync.drain()
                si2 = d2.ins.sync_info or mybir.SyncInfo(on_wait=[], on_update=[])
                si2.on_wait = [w]
                d2.ins.sync_info = si2
        self.nc.all_engine_barrier()
        assert self.sems is not None
        popped = self.nc._tile_sem_poison_stack.pop()
        assert popped is self._sem_poison
        self.nc.clear_and_free_semaphores(list(self.sems.allocated().values()))
        self.nc.all_engine_barrier()

    tile.TileContext._drain_and_barrier = _patched
    tile.TileContext._drain_split_patched = True


def _split_multiwaits(nc):
    """This container's walrus codegen accepts at most ONE semaphore wait
    per instruction. Hoist extra waits onto NoOps inserted just before the
    offending instruction on the same engine (engine streams are in-order,
    so waiting earlier on the same queue is equivalent)."""
    import concourse.mybir as mybir

    ctr = [0]
    for fn in nc.m.functions:
        for bb in fn.blocks:
            changed = False
            new_insts = []
            for inst in bb.instructions:
                si = inst.sync_info
                if si is not None and len(si.on_wait) > 1:
                    waits = list(si.on_wait)
                    for w in waits[:-1]:
                        nop = mybir.InstNoOp(
                            name=f"I-mwsplit-{ctr[0]}", ins=[], outs=[])
                        ctr[0] += 1
                        nop.engine = inst.engine
                        nop.sync_info = mybir.SyncInfo(
                            on_wait=[w], on_update=[])
                        new_insts.append(nop)
                    si.on_wait = [waits[-1]]
                    inst.sync_info = si
                    changed = True
                new_insts.append(inst)
            if changed:
                bb.instructions = new_insts


def build_nc(has_bun=False, has_bo=False):
    import concourse.bass as bass
    import concourse.mybir as mybir
    import concourse.tile as tile

    _apply_tile_patch()
    dt = mybir.dt
    nc = bass.Bass()

    e_d = nc.declare_dram_parameter("e_pad", [V, EP], dt.bfloat16, isOutput=False)
    idx_d = nc.declare_dram_parameter("idx32", [128, NBT // 128], dt.int32, isOutput=False)
    mid_d = nc.declare_dram_parameter("midisT", [M_DIM, NBT], dt.bfloat16, isOutput=False)
    om_d = nc.declare_dram_parameter("onesmask", [2, NBT], dt.bfloat16, isOutput=False)
    wm_d = nc.declare_dram_parameter("wm", [M_DIM, 3 * 128], dt.bfloat16, isOutput=False)
    wxp_d = nc.declare_dram_parameter("wxp", [128, 9 * H], dt.bfloat16, isOutput=False)
    up_d = nc.declare_dram_parameter("up", [128, 6 * H], dt.bfloat16, isOutput=False)
    bun_d = nc.declare_dram_parameter("bun2", [128, 2], dt.float32, isOutput=False)
    id_d = nc.declare_dram_parameter("ident", [128, 128], dt.bfloat16, isOutput=False)
    wo_d = nc.declare_dram_parameter("wo", [128, 2 * VSH], dt.bfloat16, isOutput=False)
    bo_d = nc.declare_dram_parameter("bo_b", [128, VSH], dt.float32, isOutput=False)
    out_d = nc.declare_dram_parameter("out", [B, T, VSH], dt.float32, isOutput=True)

    KC2 = KX - 256                                          # 46 rows in chunk 2
    KCH = [128, 128, KC2]

    with tile.TileContext(nc) as tc, ExitStack() as ctx:
        singles = ctx.enter_context(tc.tile_pool(name="singles", bufs=1))
        big = ctx.enter_context(tc.tile_pool(name="big", bufs=1))
        xep = ctx.enter_context(tc.tile_pool(name="xep", bufs=3))
        midp = ctx.enter_context(tc.tile_pool(name="midp", bufs=2))
        work = ctx.enter_context(tc.tile_pool(name="work", bufs=3))
        loutp = ctx.enter_context(tc.tile_pool(name="lout", bufs=2))
        pmm = ctx.enter_context(tc.tile_pool(name="pmm", bufs=2, space="PSUM"))
        prec = ctx.enter_context(tc.tile_pool(name="prec", bufs=2, space="PSUM"))

        f32, b16 = dt.float32, dt.bfloat16

        # ---- resident tensors ----
        idx_s = singles.tile([128, NBT // 128], dt.int32)
        nc.sync.dma_start(out=idx_s[:], in_=idx_d[:])
        wm_s = singles.tile([M_DIM, 3 * 128], b16)
        nc.sync.dma_start(out=wm_s[:], in_=wm_d[:])
        wxp_s = singles.tile([128, 9 * H], b16)
        nc.sync.dma_start(out=wxp_s[:], in_=wxp_d[:])
        up_s = singles.tile([128, 6 * H], b16)
        nc.sync.dma_start(out=up_s[:], in_=up_d[:])
        id_s = singles.tile([128, 128], b16)
        nc.sync.dma_start(out=id_s[:], in_=id_d[:])
        wo_s = singles.tile([128, 2 * VSH], b16)
        nc.sync.dma_start(out=wo_s[:], in_=wo_d[:])
        bun_s = singles.tile([128, 2], f32)
        if has_bun:
            nc.sync.dma_start(out=bun_s[:], in_=bun_d[:])
        bo_s = singles.tile([128, VSH], f32)
        if has_bo:
            nc.sync.dma_start(out=bo_s[:], in_=bo_d[:])

        gxT = big.tile([128, 6, NBT], b16)                  # 96KB/part
        seqT = big.tile([128, 2, T, 32], b16)               # 32KB/part
        h0 = singles.tile([128, 2, 32], b16)
        nc.vector.memset(h0[:], 0.0)

        def emit_colgroup(cg):
            """x-stage + gx-stage for token col-group cg (512 tokens)."""
            c0 = cg * CG
            # gather E rows (token-row layout), then xbar-transpose into
            # xe[p, chunk, tok]
            xe = xep.tile([128, 3, CG], b16, tag="xe")
            for gt in range(CG // 128):
                g = midp.tile([128, EP], b16, tag="gath")
                nc.gpsimd.indirect_dma_start(
                    out=g[:],
                    out_offset=None,
                    in_=e_d[:],
                    in_offset=bass.IndirectOffsetOnAxis(
                        ap=idx_s[:, cg * 4 + gt: cg * 4 + gt + 1], axis=0),
                )
                for c in range(3):
                    nc.sync.dma_start_transpose(
                        out=xe[:, c, gt * 128:(gt + 1) * 128],
                        in_=g[:, c * 128:(c + 1) * 128])
            # midis projection, accumulate into xe in place
            mt = midp.tile([M_DIM, CG], b16, tag="mid")
            nc.sync.dma_start(out=mt[:], in_=mid_d[:, c0:c0 + CG])
            for c in range(3):
                mc = [128, 128, E_DIM - 256][c]
                ps = pmm.tile([128, CG], f32, tag="pmm")
                nc.tensor.matmul(ps[:mc, :], wm_s[:, c * 128:c * 128 + mc], mt[:],
                                 start=True, stop=True)
                nc.vector.tensor_add(xe[:mc, c, :], ps[:mc, :], xe[:mc, c, :])
            # ones + mask rows (chunk 2, rows 44/45)
            nc.sync.dma_start(out=xe[E_DIM - 256:KC2, 2, :], in_=om_d[:, c0:c0 + CG])
            # gx = Wx'.T @ x
            for m in range(6):
                pg = pmm.tile([128, CG], f32, tag="pmm")
                for c in range(3):
                    nc.tensor.matmul(
                        pg[:, :],
                        wxp_s[:KCH[c], c * 3 * H + m * 128: c * 3 * H + m * 128 + 128],
                        xe[:KCH[c], c, :],
                        start=(c == 0), stop=(c == 2))
                if m % 2 == 0:
                    nc.vector.tensor_copy(gxT[:, m, c0:c0 + CG], pg[:, :])
                else:
                    nc.scalar.copy(gxT[:, m, c0:c0 + CG], pg[:, :])

        NS = [512, 512, VSH - 1024]
        lt_cur = [None]

        def emit_logits_piece(tb, ns):
            """One N-split of the output GEMM for t-block tb."""
            t0 = tb * 4
            if ns == 0:
                lt_cur[0] = loutp.tile([128, VSH], f32, tag="lt", name="lt")
            lt = lt_cur[0]
            base = seqT[:, 0, 0, :]
            n0 = ns * 512
            pl = pmm.tile([128, 512], f32, tag="pmm")
            for cc in range(2):
                lhsT = bass.AP(
                    tensor=base.tensor,
                    offset=base.offset + (cc * T * 32 + t0 * 32),
                    ap=[base.ap[0], [1, 128]])
                nc.tensor.matmul(pl[:, :NS[ns]], lhsT,
                                 wo_s[:, cc * VSH + n0: cc * VSH + n0 + NS[ns]],
                                 start=(cc == 0), stop=(cc == 1))
            if ns == 1:
                nc.vector.tensor_copy(lt[:, n0:n0 + NS[ns]], pl[:, :NS[ns]])
            else:
                nc.scalar.copy(lt[:, n0:n0 + NS[ns]], pl[:, :NS[ns]])
            if ns == 2:
                if has_bo:
                    nc.vector.tensor_add(lt[:], lt[:], bo_s[:])
                for dti in range(4):
                    nc.sync.dma_start(
                        out=out_d[:, t0 + dti, :],
                        in_=lt[dti * 32:(dti + 1) * 32, :])

        # ---- x/gx pipeline prologue: 2 col groups ahead ----
        emit_colgroup(0)
        emit_colgroup(1)

        # ---- recurrence ----
        lpieces = []
        for t in range(T):
            if t % 16 == 0 and t // 16 + 2 < NCG:
                emit_colgroup(t // 16 + 2)
            ts32 = slice(t * 32, (t + 1) * 32)
            hprev = h0[:] if t == 0 else seqT[:, :, t - 1, :]
            pr = prec.tile([128, 2, 32], f32, tag="pr")
            pz = prec.tile([128, 2, 32], f32, tag="pz")
            pn = prec.tile([128, 2, 32], f32, tag="pn")
            # r-gate first (the serial chain needs r earliest), then n, then z
            nc.tensor.matmul(pr[:, :, :], id_s[:], gxT[:, 2:4, ts32],
                             start=True, stop=False, skip_group_check=True)
            for j in range(2):
                for kc in range(2):
                    nc.tensor.matmul(
                        pr[:, j, :],
                        up_s[:, kc * 3 * H + (2 + j) * 128: kc * 3 * H + (3 + j) * 128],
                        hprev[:, kc, :],
                        start=False, stop=(kc == 1), skip_group_check=True)
            for j in range(2):      # n0 n1
                for kc in range(2):
                    nc.tensor.matmul(
                        pn[:, j, :],
                        up_s[:, kc * 3 * H + (4 + j) * 128: kc * 3 * H + (5 + j) * 128],
                        hprev[:, kc, :],
                        start=(kc == 0), stop=(kc == 1), skip_group_check=True)
            nc.tensor.matmul(pz[:, :, :], id_s[:], gxT[:, 0:2, ts32],
                             start=True, stop=False, skip_group_check=True)
            for j in range(2):
                for kc in range(2):
                    nc.tensor.matmul(
                        pz[:, j, :],
                        up_s[:, kc * 3 * H + j * 128: kc * 3 * H + (j + 1) * 128],
                        hprev[:, kc, :],
                        start=False, stop=(kc == 1), skip_group_check=True)
            r_t = work.tile([128, 2, 32], b16, tag="r")
            nc.scalar.activation(r_t[:], pr[:], mybir.ActivationFunctionType.Sigmoid)
            s_t = work.tile([128, 2, 32], b16, tag="s")
            nc.scalar.activation(s_t[:], pz[:], mybir.ActivationFunctionType.Sigmoid)
            q = work.tile([128, 2, 32], b16, tag="q")
            if has_bun:
                for j in range(2):
                    nc.vector.scalar_tensor_tensor(
                        q[:, j, :], pn[:, j, :], bun_s[:, j:j + 1], r_t[:, j, :],
                        op0=mybir.AluOpType.add, op1=mybir.AluOpType.mult)
            else:
                nc.vector.tensor_mul(q[:], pn[:], r_t[:])
            p = work.tile([128, 2, 32], b16, tag="p")
            nc.vector.tensor_add(p[:], q[:], gxT[:, 4:6, ts32])
            n_t = work.tile([128, 2, 32], b16, tag="n")
            nc.scalar.activation(n_t[:], p[:], mybir.ActivationFunctionType.Tanh)
            d = work.tile([128, 2, 32], b16, tag="d")
            nc.vector.tensor_sub(d[:], hprev, n_t[:])
            e = work.tile([128, 2, 32], b16, tag="e")
            nc.vector.tensor_mul(e[:], s_t[:], d[:])
            nc.vector.tensor_sub(seqT[:, :, t, :], hprev, e[:])
            # pipeline the output GEMM: one N-split piece per step
            if t % 4 == 3:
                tb = t // 4
                lpieces.extend([(tb, 0), (tb, 1), (tb, 2)])
            if lpieces:
                emit_logits_piece(*lpieces.pop(0))
        while lpieces:
            emit_logits_piece(*lpieces.pop(0))

    _split_multiwaits(nc)
    return nc


_BUILD_CACHE = {}


def _get_built(has_bun, has_bo):
    key = (has_bun, has_bo)
    if key not in _BUILD_CACHE:
        _BUILD_CACHE[key] = build_nc(has_bun, has_bo)
    return _BUILD_CACHE[key]


def make_in_maps(prep):
    maps = []
    for c in range(NCORES):
        m = dict(e_pad=prep["e_pad"], idx32=prep["idx32"],
                 midisT=prep["midisT"], onesmask=prep["onesmask"],
                 wm=prep["wm"], wxp=prep["wxp"], up=prep["up"],
                 bun2=prep["bun2"], ident=prep["ident"],
                 wo=prep["wo_cores"][c],
                 bo_b=np.broadcast_to(
                     prep["bo_f"][c * VSH:(c + 1) * VSH].astype(np.float32),
                     (128, VSH)).copy())
        maps.append(m)
    return maps


_EXEC_CACHE = {}


def _get_executor(nc):
    """Build (once) a reusable sharded PJRT executable for `nc` across the
    8 cores. run_bass_kernel_spmd's axon path re-jits on every call; caching
    the jitted function makes repeated kernel() calls cheap."""
    key = id(nc)
    if key in _EXEC_CACHE:
        return _EXEC_CACHE[key]
    import jax
    from jax.sharding import Mesh, PartitionSpec
    from jax.experimental.shard_map import shard_map
    from concourse import bass2jax
    import concourse.mybir as mybir

    bass2jax.install_neuronx_cc_hook()
    in_names, out_names, out_avals, zero_outs = [], [], [], []
    for alloc in nc.m.functions[0].allocations:
        if not isinstance(alloc, mybir.MemoryLocationSet):
            continue
        name = alloc.memorylocations[0].name
        if alloc.kind == "ExternalInput":
            if nc.partition_id_tensor is None or name != nc.partition_id_tensor.name:
                in_names.append(name)
        elif alloc.kind == "ExternalOutput":
            shape = tuple(alloc.tensor_shape)
            dtype = mybir.dt.np(alloc.dtype)
            out_names.append(name)
            out_avals.append(jax.core.ShapedArray(shape, dtype))
            zero_outs.append(np.zeros(shape, dtype))
    n_params = len(in_names)

    partition_name = (nc.partition_id_tensor.name
                      if nc.partition_id_tensor else None)
    bind_in_names = list(in_names) + list(out_names)
    if partition_name is not None:
        bind_in_names.append(partition_name)

    def _body(*args):
        operands = list(args)
        if partition_name is not None:
            operands.append(bass2jax.partition_id_tensor())
        outs = bass2jax._bass_exec_p.bind(
            *operands,
            out_avals=tuple(out_avals),
            in_names=tuple(bind_in_names),
            out_names=tuple(out_names),
            lowering_input_output_aliases=(),
            sim_require_finite=True,
            sim_require_nnan=True,
            nc=nc)
        return tuple(outs)

    devices = jax.devices()[:NCORES]
    mesh = Mesh(np.asarray(devices), ("core",))
    in_specs = (PartitionSpec("core"),) * (n_params + len(out_avals))
    out_specs = (PartitionSpec("core"),) * len(out_avals)
    sharded = jax.jit(
        shard_map(_body, mesh=mesh, in_specs=in_specs, out_specs=out_specs,
                  check_rep=False),
        keep_unused=True)

    def run(in_maps):
        per_core = [[np.asarray(m[name]) for name in in_names] for m in in_maps]
        concat_in = [np.concatenate([per_core[c][i] for c in range(NCORES)], axis=0)
                     for i in range(n_params)]
        concat_zeros = [np.zeros((NCORES * z.shape[0], *z.shape[1:]), z.dtype)
                        for z in zero_outs]
        outs = sharded(*concat_in, *concat_zeros)
        jax.block_until_ready(outs)
        return [
            {name: np.asarray(outs[i]).reshape(NCORES, *out_avals[i].shape)[c]
             for i, name in enumerate(out_names)}
            for c in range(NCORES)
        ]

    _EXEC_CACHE[key] = run
    return run


def kernel(words, midis, E, Wm, bm, Wx, bx, U, bu, Wo, bo):
    prep = _host_prep(words, midis, E, Wm, bm, Wx, bx, U, bu, Wo, bo)
    nc = _get_built(prep["has_bun"], prep["has_bo"])
    results = _get_executor(nc)(make_in_maps(prep))
    return np.concatenate([r["out"] for r in results], axis=2)

